# revision 6
# baseline (speedup 1.0000x reference)
"""Linformer self-attention (degenerate-einsum variant) on 8 TRN2 NeuronCores.

Math (from the reference):
  k_proj[b,h,k,d] = E[k,d] * S_k[b,h*64+d]  where S_k[b,:] = (sum_n x[b,n,:]) @ Wk.T
  attn = softmax( (q * S_k) @ E.T / 8 )  per (b, head)
  out  = (attn @ (F * S_v)) restored to (B,N,D), then @ Wo.T + bo

Sharding: core c = (batch b = c//2, sequence half = c%2); each core computes a
(2048, 1024) slice. Device does Q-proj, per-head logits (row-tiled matmuls,
heads packed on partition halves), softmax, PE transposes, col-tiled ohat and
the Wo matmul in bf16. Host folds S_k/S_v into ehat/fhat, pre-rounds fp32r,
adds the output bias, and casts the bf16 result back to f32.
"""

import numpy as np
import ml_dtypes

import concourse.bass as bass
import concourse.bacc as bacc
import concourse.tile as tile
import concourse.mybir as mybir
import concourse.bass_utils as bass_utils

B, N, D = 4, 4096, 1024
H, HD, KP = 16, 64, 256
NCORES = 8
NH = N // 2          # rows per core = 2048
HBLK = 256           # half-block rows
NHB = NH // HBLK     # 8 half-blocks
F32 = mybir.dt.float32
F32R = mybir.dt.float32r
BF16 = mybir.dt.bfloat16

_CACHE = {}


def _round_fp32r(a: np.ndarray) -> np.ndarray:
    """Round-to-nearest-even fp32 -> fp32r (11 explicit mantissa bits)."""
    b = np.ascontiguousarray(a, dtype=np.float32).view(np.uint32)
    low = b & np.uint32(0xFFF)
    bit12 = (b >> np.uint32(12)) & np.uint32(1)
    up = (low > 0x800) | ((low == 0x800) & (bit12 == 1))
    r = (b & np.uint32(0xFFFFF000)) + (up.astype(np.uint32) << np.uint32(12))
    return r.view(np.float32)


def _build():
    nc = bacc.Bacc("TRN2", target_bir_lowering=False, debug=False, num_devices=NCORES)

    xT_d = nc.dram_tensor("xT", [D, NH], F32R, kind="ExternalInput").ap()
    wqT_d = nc.dram_tensor("wqT", [D, D], F32R, kind="ExternalInput").ap()
    woT_d = nc.dram_tensor("woT", [D, D], BF16, kind="ExternalInput").ap()
    ehat_d = nc.dram_tensor("ehat", [128, 8, KP], F32R, kind="ExternalInput").ap()
    fhat_d = nc.dram_tensor("fhat", [128, 8, 2, 2, HD], BF16, kind="ExternalInput").ap()
    ident_d = nc.dram_tensor("ident", [128, 128], BF16, kind="ExternalInput").ap()
    out_d = nc.dram_tensor("out", [NH, D], BF16, kind="ExternalOutput").ap()

    with tile.TileContext(nc) as tc:
        with (
            tc.tile_pool(name="wq", bufs=1) as wq_pool,
            tc.tile_pool(name="wo", bufs=1) as wo_pool,
            tc.tile_pool(name="const", bufs=1) as const_pool,
            tc.tile_pool(name="xt", bufs=10) as xt_pool,
            tc.tile_pool(name="qt", bufs=14) as qt_pool,
            tc.tile_pool(name="estat", bufs=10) as stat_pool,
            tc.tile_pool(name="ep", bufs=8) as e_pool,
            tc.tile_pool(name="pp", bufs=8) as p_pool,
            tc.tile_pool(name="pts", bufs=18) as pts_pool,
            tc.tile_pool(name="ohat", bufs=5) as ohat_pool,
            tc.tile_pool(name="osb", bufs=3) as out_pool,
            tc.tile_pool(name="qfwo", bufs=3, space=bass.MemorySpace.PSUM) as qfwo,
            tc.tile_pool(name="ap", bufs=3, space=bass.MemorySpace.PSUM) as ap_pool,
            tc.tile_pool(name="misc", bufs=2, space=bass.MemorySpace.PSUM) as misc,
        ):
            # ---- block-0 activations first: unblocks the first Q matmuls ----
            xt_state = {}

            def load_x(blk):
                xt = []
                for c in range(8):
                    t = xt_pool.tile([128, 512], F32R, tag="xt", name=f"xt{c}")
                    nc.sync.dma_start(
                        t[:], xT_d[c * 128:(c + 1) * 128, blk * 512:(blk + 1) * 512]
                    )
                    xt.append(t)
                xt_state[blk] = xt

            load_x(0)

            wq_sb = []
            wo_sb = []
            for c in range(8):
                t = wq_pool.tile([128, D], F32R, tag=f"wq{c}")
                nc.sync.dma_start(t[:], wqT_d[c * 128:(c + 1) * 128, :])
                wq_sb.append(t)
            ehat_sb = const_pool.tile([128, 8, KP], F32R, tag="ehat")
            nc.sync.dma_start(ehat_sb[:], ehat_d[:])
            ident_sb = const_pool.tile([128, 128], BF16, tag="ident")
            nc.sync.dma_start(ident_sb[:], ident_d[:])
            for c in range(8):
                t = wo_pool.tile([128, D], BF16, tag=f"wo{c}")
                nc.sync.dma_start(t[:], woT_d[c * 128:(c + 1) * 128, :])
                wo_sb.append(t)
            fhat_sb = const_pool.tile([128, 8, 2, 2, HD], BF16, tag="fhat")
            nc.sync.dma_start(fhat_sb[:], fhat_d[:])

            p_state = {}

            def stage_a(hb):
                blk = hb // 2

                def q_chunks(b, cos):
                    if b not in xt_state:
                        load_x(b)
                    xt = xt_state[b]
                    qt = p_state.setdefault((b, "qt"), {})
                    for co in cos:
                        qp = qfwo.tile([128, 512], F32, tag="qf", name=f"qp{co}")
                        for ck in range(8):
                            nc.tensor.matmul(
                                qp[:],
                                wq_sb[ck][:, co * 128:(co + 1) * 128],
                                xt[ck][:],
                                start=(ck == 0),
                                stop=(ck == 7),
                            )
                        q_sb = qt_pool.tile([128, 512], F32R, tag="qt", name=f"q{co}")
                        nc.scalar.copy(q_sb[:], qp[:])
                        qt[co] = q_sb
                    if max(cos) == 7:
                        xt_state.pop(b, None)
                        if b + 1 < NHB // 2:
                            load_x(b + 1)

                if hb == 0:
                    q_chunks(0, range(8))
                elif hb % 2 == 0:
                    q_chunks(blk, range(4, 8))
                else:
                    if blk + 1 < NHB // 2:
                        q_chunks(blk + 1, range(0, 4))
                qt = p_state[(blk, "qt")]

                # pts tiles for this hb: per pair [128(Kc), hh, c, s, seq]
                pts = []
                for j in range(8):
                    pts.append(
                        pts_pool.tile([128, 2, 2, 2, 128], BF16, tag="pts",
                                      name=f"pts{j}")
                    )
                p_state[(hb, "pts")] = pts

                for s in range(2):
                    sb = (hb % 2) * 2 + s
                    sl = slice(sb * 128, (sb + 1) * 128)
                    for g in range(4):
                        # heads 4g..4g+3; alpha holds (4g, 4g+2), beta (4g+1, 4g+3)
                        alpha = ap_pool.tile([128, 2, KP], F32, tag="ap", name=f"a{g}")
                        beta = ap_pool.tile([128, 2, KP], F32, tag="ap", name=f"b{g}")
                        for i in range(2):  # i: which head within the tile
                            for par in range(2):  # 0 -> even-head (alpha), 1 -> beta
                                h = 4 * g + 2 * i + par
                                dst = alpha if par == 0 else beta
                                ps = slice(par * 64, par * 64 + 64)
                                nc.tensor.matmul(
                                    dst[:, i, :],
                                    qt[h // 2][ps, sl],
                                    ehat_sb[ps, h // 2, :],
                                    start=True,
                                    stop=True,
                                )
                        negmax = stat_pool.tile([128, 4], F32, tag="negmax")
                        ssum = stat_pool.tile([128, 4], F32, tag="ssum")
                        recip = stat_pool.tile([128, 4], F32, tag="recip")
                        nc.vector.reduce_max(
                            negmax[:, 0:2], alpha[:],
                            axis=mybir.AxisListType.X, negate=True,
                        )
                        nc.vector.reduce_max(
                            negmax[:, 2:4], beta[:],
                            axis=mybir.AxisListType.X, negate=True,
                        )
                        e_tiles = {}
                        for i in range(2):
                            for par in range(2):
                                h = 4 * g + 2 * i + par
                                src = alpha if par == 0 else beta
                                col = 2 * par + i
                                e_sb = e_pool.tile([128, KP], BF16, tag="e", name=f"e{h}")
                                nc.scalar.activation(
                                    e_sb[:],
                                    src[:, i, :],
                                    mybir.ActivationFunctionType.Exp,
                                    bias=negmax[:, col:col + 1],
                                    accum_out=ssum[:, col:col + 1],
                                )
                                e_tiles[h] = (e_sb, col)
                        nc.vector.reciprocal(recip[:], ssum[:])
                        p_sbs = {}
                        for hh in range(4):
                            h = 4 * g + hh
                            e_sb, col = e_tiles[h]
                            p_sb = p_pool.tile([128, KP], BF16, tag="p", name=f"p{h}")
                            nc.vector.tensor_scalar_mul(
                                p_sb[:], e_sb[:], recip[:, col:col + 1]
                            )
                            p_sbs[h] = p_sb
                        for jj in range(2):  # pairs 2g, 2g+1
                            j = 2 * g + jj
                            # padded to 2KB so the "m" tag is shared with op
                            ptp = misc.tile([128, 2, 2, KP], BF16, tag="m",
                                            name=f"ptp{j}")
                            for hh in range(2):
                                p_sb = p_sbs[2 * j + hh]
                                for c in range(2):
                                    nc.tensor.transpose(
                                        ptp[:, hh, c, 0:128],
                                        p_sb[:, c * 128:(c + 1) * 128],
                                        ident_sb[:],
                                    )
                            if j % 2 == 0:
                                nc.scalar.copy(
                                    pts[j][:, :, :, s, :], ptp[:, :, :, 0:128]
                                )
                            else:
                                nc.vector.tensor_copy(
                                    pts[j][:, :, :, s, :], ptp[:, :, :, 0:128]
                                )

            def stage_b(hb):
                r0 = hb * HBLK
                pts = p_state.pop((hb, "pts"))
                oT = []
                for jt in range(4):  # op tile holds 2 pairs
                    op_ = misc.tile([128, 2, KP], F32, tag="m", name=f"op{jt}")
                    for slot in range(2):
                        j = 2 * jt + slot
                        for hh in range(2):
                            for c in range(2):
                                nc.tensor.matmul(
                                    op_[hh * 64:(hh + 1) * 64, slot, :],
                                    fhat_sb[:, j, hh, c, :],
                                    pts[j][:, hh, c, :, :],
                                    start=(c == 0),
                                    stop=(c == 1),
                                )
                    ot = ohat_pool.tile([128, 2, KP], BF16, tag="oT",
                                        name=f"oT{jt}")
                    nc.vector.tensor_copy(ot[:], op_[:])
                    oT.append(ot)
                for s in range(2):
                    for half in range(2):
                        fp_ = qfwo.tile([128, 512], F32, tag="qf", name=f"fp{s}{half}")
                        for j in range(8):
                            nc.tensor.matmul(
                                fp_[:],
                                oT[j // 2][:, j % 2, s * 128:(s + 1) * 128],
                                wo_sb[j][:, half * 512:(half + 1) * 512],
                                start=(j == 0),
                                stop=(j == 7),
                            )
                        o_sb = out_pool.tile([128, 512], BF16, tag="osb",
                                             name=f"o{s}{half}")
                        nc.vector.tensor_copy(o_sb[:], fp_[:])
                        nc.sync.dma_start(
                            out_d[r0 + s * 128:r0 + (s + 1) * 128,
                                  half * 512:(half + 1) * 512],
                            o_sb[:],
                        )

            for hb in range(NHB + 1):
                if hb >= 1:
                    stage_b(hb - 1)
                    if hb % 2 == 0:
                        p_state.pop(((hb - 1) // 2, "qt"), None)
                if hb < NHB:
                    stage_a(hb)

    nc.compile()
    return nc


def _prep_inputs(x, Wq, Wk, Wv, E, F, Wo, bo):
    x = np.asarray(x, dtype=np.float32)
    Wq = np.asarray(Wq, dtype=np.float32)
    Wk = np.asarray(Wk, dtype=np.float32)
    Wv = np.asarray(Wv, dtype=np.float32)
    E = np.asarray(E, dtype=np.float32)
    F_ = np.asarray(F, dtype=np.float32)
    Wo = np.asarray(Wo, dtype=np.float32)

    xsum = x.sum(axis=1)                     # (B, D)
    S_k = xsum @ Wk.T                        # (B, D)
    S_v = xsum @ Wv.T                        # (B, D)

    wqT = _round_fp32r(np.ascontiguousarray(Wq.T))
    woT = np.ascontiguousarray(Wo.T).astype(ml_dtypes.bfloat16)
    ident = np.eye(128, dtype=ml_dtypes.bfloat16)

    in_maps = []
    for core in range(NCORES):
        b, half = core // 2, core % 2
        xs = x[b, half * NH:(half + 1) * NH, :]
        xT = _round_fp32r(np.ascontiguousarray(xs.T))    # (D, NH)

        # ehat: per pair j, partitions 0:64 even head, 64:128 odd head
        ehat = np.zeros((128, 8, KP), dtype=np.float32)
        for h in range(H):
            sk = S_k[b, h * HD:(h + 1) * HD]             # (64,)
            j, hh = h // 2, h % 2
            ehat[hh * 64:hh * 64 + 64, j, :] = (E.T * sk[:, None]) / 8.0
        ehat = _round_fp32r(ehat)

        # fhat: dense per head, [Kchunk 128, pair, head-in-pair, chunk, 64]
        fhat = np.zeros((128, 8, 2, 2, HD), dtype=np.float32)
        for h in range(H):
            sv = S_v[b, h * HD:(h + 1) * HD]
            fh = F_ * sv[None, :]                        # (KP, 64)
            j, hh = h // 2, h % 2
            for c in range(2):
                fhat[:, j, hh, c, :] = fh[c * 128:(c + 1) * 128, :]
        fhat = fhat.astype(ml_dtypes.bfloat16)

        in_maps.append({
            "xT": xT, "wqT": wqT, "woT": woT, "ehat": ehat,
            "fhat": fhat, "ident": ident,
        })
    return in_maps


def _run(inputs: dict, trace: bool = False, tmpdir: str | None = None):
    if "nc" not in _CACHE:
        _CACHE["nc"] = _build()
    nc = _CACHE["nc"]
    bo = np.asarray(inputs["bo"], dtype=np.float32)
    in_maps = _prep_inputs(**inputs)
    res = bass_utils.run_bass_kernel_spmd(
        nc, in_maps, core_ids=list(range(NCORES)), trace=trace, tmpdir=tmpdir
    )
    out = np.empty((B, N, D), dtype=np.float32)
    for core in range(NCORES):
        b, half = core // 2, core % 2
        out[b, half * NH:(half + 1) * NH, :] = (
            res.results[core]["out"].astype(np.float32) + bo[None, :]
        )
    return out, res


def kernel(**inputs) -> np.ndarray:
    out, _ = _run(inputs)
    return out


# revision 9
# speedup vs baseline: 1.1957x; 1.1957x over previous
"""Linformer self-attention (degenerate-einsum variant) on 8 TRN2 NeuronCores.

Math (from the reference):
  k_proj[b,h,k,d] = E[k,d] * S_k[b,h*64+d]  where S_k[b,:] = (sum_n x[b,n,:]) @ Wk.T
  attn = softmax( (q * S_k) @ E.T / 8 )  per (b, head)
  out  = (attn @ (F * S_v)) restored to (B,N,D), then @ Wo.T + bo

Sharding: core c = (batch b = c//2, sequence half = c%2); each core computes a
(2048, 1024) slice. Host folds S_k/S_v into per-head E-hat (fp32r, block
diagonal) and F-hat (bf16), rounds fp32r operands, adds the output bias and
casts the device's bf16 result back to f32.

Emission interleaves the softmax-dependent PE work (transposes of the previous
half-block) between logits groups so the tensor engine never drains while the
DVE/ACT softmax chain catches up (keeps the PE HAM clock warm).
"""

import numpy as np
import ml_dtypes

import concourse.bass as bass
import concourse.bacc as bacc
import concourse.tile as tile
import concourse.mybir as mybir
import concourse.bass_utils as bass_utils

B, N, D = 4, 4096, 1024
H, HD, KP = 16, 64, 256
NCORES = 8
NH = N // 2          # rows per core = 2048
HBLK = 256           # half-block rows
NHB = NH // HBLK     # 8 half-blocks
F32 = mybir.dt.float32
F32R = mybir.dt.float32r
BF16 = mybir.dt.bfloat16

_CACHE = {}


def _round_fp32r(a: np.ndarray) -> np.ndarray:
    """Round-to-nearest-even fp32 -> fp32r (11 explicit mantissa bits)."""
    b = np.ascontiguousarray(a, dtype=np.float32).view(np.uint32)
    low = b & np.uint32(0xFFF)
    bit12 = (b >> np.uint32(12)) & np.uint32(1)
    up = (low > 0x800) | ((low == 0x800) & (bit12 == 1))
    r = (b & np.uint32(0xFFFFF000)) + (up.astype(np.uint32) << np.uint32(12))
    return r.view(np.float32)


def _build():
    nc = bacc.Bacc("TRN2", target_bir_lowering=False, debug=False, num_devices=NCORES)

    xT_d = nc.dram_tensor("xT", [D, NH], F32R, kind="ExternalInput").ap()
    wqT_d = nc.dram_tensor("wqT", [D, D], F32R, kind="ExternalInput").ap()
    woT_d = nc.dram_tensor("woT", [D, D], BF16, kind="ExternalInput").ap()
    ehat_d = nc.dram_tensor("ehat", [128, 8, 2 * KP], F32R, kind="ExternalInput").ap()
    fhat_d = nc.dram_tensor("fhat", [128, 8, 2, 2, 128], BF16, kind="ExternalInput").ap()
    ident_d = nc.dram_tensor("ident", [128, 128], BF16, kind="ExternalInput").ap()
    out_d = nc.dram_tensor("out", [NH, D], BF16, kind="ExternalOutput").ap()

    with tile.TileContext(nc) as tc:
        with (
            tc.tile_pool(name="wq", bufs=1) as wq_pool,
            tc.tile_pool(name="wo", bufs=1) as wo_pool,
            tc.tile_pool(name="const", bufs=1) as const_pool,
            tc.tile_pool(name="xt", bufs=10) as xt_pool,
            tc.tile_pool(name="qt", bufs=14) as qt_pool,
            tc.tile_pool(name="estat", bufs=8) as stat_pool,
            tc.tile_pool(name="ep", bufs=8) as e_pool,
            tc.tile_pool(name="pp", bufs=40) as p_pool,
            tc.tile_pool(name="pt", bufs=20) as pt_pool,
            tc.tile_pool(name="ohat", bufs=10) as ohat_pool,
            tc.tile_pool(name="osb", bufs=3) as out_pool,
            tc.tile_pool(name="qfpsum", bufs=2, space=bass.MemorySpace.PSUM) as qfpsum,
            tc.tile_pool(name="apsum", bufs=3, space=bass.MemorySpace.PSUM) as apsum,
            tc.tile_pool(name="ppsum", bufs=2, space=bass.MemorySpace.PSUM) as ppsum,
            tc.tile_pool(name="opsum", bufs=1, space=bass.MemorySpace.PSUM) as opsum,
        ):
            xt_state = {}

            def load_x(blk):
                xt = []
                for c in range(8):
                    t = xt_pool.tile([128, 512], F32R, tag="xt", name=f"xt{c}")
                    nc.sync.dma_start(
                        t[:], xT_d[c * 128:(c + 1) * 128, blk * 512:(blk + 1) * 512]
                    )
                    xt.append(t)
                xt_state[blk] = xt

            load_x(0)

            wq_sb = []
            wo_sb = []
            for c in range(8):
                t = wq_pool.tile([128, D], F32R, tag=f"wq{c}")
                nc.sync.dma_start(t[:], wqT_d[c * 128:(c + 1) * 128, :])
                wq_sb.append(t)
            ehat_sb = const_pool.tile([128, 8, 2 * KP], F32R, tag="ehat")
            nc.sync.dma_start(ehat_sb[:], ehat_d[:])
            ident_sb = const_pool.tile([128, 128], BF16, tag="ident")
            nc.sync.dma_start(ident_sb[:], ident_d[:])
            for c in range(8):
                t = wo_pool.tile([128, D], BF16, tag=f"wo{c}")
                nc.sync.dma_start(t[:], woT_d[c * 128:(c + 1) * 128, :])
                wo_sb.append(t)
            fhat_sb = const_pool.tile([128, 8, 2, 2, 128], BF16, tag="fhat")
            nc.sync.dma_start(fhat_sb[:], fhat_d[:])

            p_state = {}

            # ---------- stage-a units ----------
            def q_unit(b, co):
                """Q projection chunk co of block b -> qt tile."""
                xt = xt_state[b]
                qt = p_state.setdefault((b, "qt"), {})
                qp = qfpsum.tile([128, 512], F32, tag="qf", name=f"qp{co}")
                for ck in range(8):
                    nc.tensor.matmul(
                        qp[:],
                        wq_sb[ck][:, co * 128:(co + 1) * 128],
                        xt[ck][:],
                        start=(ck == 0),
                        stop=(ck == 7),
                    )
                q_sb = qt_pool.tile([128, 512], F32R, tag="qt", name=f"q{co}")
                nc.scalar.copy(q_sb[:], qp[:])
                qt[co] = q_sb

            def softmax_unit(hb, s, g):
                """Logits + softmax for heads 4g..4g+3 of row-block (hb, s)."""
                blk = hb // 2
                qt = p_state[(blk, "qt")]
                sb = (hb % 2) * 2 + s
                aps = []
                negmax = stat_pool.tile([128, 4], F32, tag="negmax")
                ssum = stat_pool.tile([128, 4], F32, tag="ssum")
                for jj in range(2):
                    j = 2 * g + jj
                    ap_ = apsum.tile([128, 2 * KP], F32, tag="ap", name=f"ap{j}")
                    nc.tensor.matmul(
                        ap_[:],
                        qt[j][:, sb * 128:(sb + 1) * 128],
                        ehat_sb[:, j, :],
                        start=True,
                        stop=True,
                    )
                    aps.append(ap_)
                    nc.vector.reduce_max(
                        negmax[:, 2 * jj:2 * jj + 2],
                        ap_[:].rearrange("p (c k) -> p c k", c=2),
                        axis=mybir.AxisListType.X, negate=True,
                    )
                e_tiles = []
                for hh in range(4):
                    h = 4 * g + hh
                    e_sb = e_pool.tile([128, KP], BF16, tag="e", name=f"e{h}")
                    nc.scalar.activation(
                        e_sb[:],
                        aps[hh // 2][:, (hh % 2) * KP:(hh % 2 + 1) * KP],
                        mybir.ActivationFunctionType.Exp,
                        bias=negmax[:, hh:hh + 1], accum_out=ssum[:, hh:hh + 1],
                    )
                    e_tiles.append(e_sb)
                recip = stat_pool.tile([128, 4], F32, tag="recip")
                nc.vector.reciprocal(recip[:], ssum[:])
                for hh in range(4):
                    h = 4 * g + hh
                    p_sb = p_pool.tile([128, KP], BF16, tag="p", name=f"p{h}")
                    nc.vector.tensor_scalar_mul(
                        p_sb[:], e_tiles[hh][:], recip[:, hh:hh + 1]
                    )
                    p_state[(hb, s, h)] = p_sb

            # ---------- stage-b units (for half-block hb) ----------
            def t_unit(hb, s, h2, pts):
                """Transpose p tiles of heads h2, h2+1 into pts (pair of heads)."""
                for h in (h2, h2 + 1):
                    p_sb = p_state.pop((hb, s, h))
                    ptp = ppsum.tile([128, KP], BF16, tag="ptp", name=f"ptp{h}")
                    for c in range(2):
                        nc.tensor.transpose(
                            ptp[:, c * 128:(c + 1) * 128],
                            p_sb[:, c * 128:(c + 1) * 128],
                            ident_sb[:],
                        )
                    eng_copy = nc.scalar.copy if h % 2 == 0 else nc.vector.tensor_copy
                    eng_copy(
                        pts[h][:, :, s * 128:(s + 1) * 128],
                        ptp[:].rearrange("p (c r) -> p c r", c=2),
                    )

            def o_unit(j, pts, ohatT):
                op_ = opsum.tile([128, HBLK], F32, tag="op", name=f"op{j}")
                first = True
                for hh in range(2):
                    for c in range(2):
                        nc.tensor.matmul(
                            op_[:],
                            fhat_sb[:, j, hh, c, :],
                            pts[2 * j + hh][:, c, :],
                            start=first,
                            stop=(hh == 1 and c == 1),
                        )
                        first = False
                oT = ohat_pool.tile([128, HBLK], BF16, tag="ohatT", name=f"oT{j}")
                nc.vector.tensor_copy(oT[:], op_[:])
                ohatT.append(oT)

            def w_unit(hb, s, half, ohatT):
                r0 = hb * HBLK
                fp_ = qfpsum.tile([128, 512], F32, tag="qf", name=f"fp{s}{half}")
                for j in range(8):
                    nc.tensor.matmul(
                        fp_[:],
                        ohatT[j][:, s * 128:(s + 1) * 128],
                        wo_sb[j][:, half * 512:(half + 1) * 512],
                        start=(j == 0),
                        stop=(j == 7),
                    )
                o_sb = out_pool.tile([128, 512], BF16, tag="osb", name=f"o{s}{half}")
                nc.vector.tensor_copy(o_sb[:], fp_[:])
                nc.sync.dma_start(
                    out_d[r0 + s * 128:r0 + (s + 1) * 128,
                          half * 512:(half + 1) * 512],
                    o_sb[:],
                )

            # ---------- interleaved emission ----------
            def emit_hb(hb):
                """Emit stage_a(hb) interleaved with stage_b(hb-1)."""
                blk = hb // 2
                # stage-a Q units for this hb
                qunits = []
                if hb == 0:
                    qunits = [(0, co) for co in range(8)]
                elif hb % 2 == 0:
                    qunits = [(blk, co) for co in range(4, 8)]
                elif blk + 1 < NHB // 2:
                    if (blk + 1) not in xt_state:
                        load_x(blk + 1)
                    qunits = [(blk + 1, co) for co in range(0, 4)]

                # stage-b units for hb-1
                hbp = hb - 1
                tunits = []
                pts = None
                if hbp >= 0:
                    pts = []
                    for h in range(H):
                        pts.append(
                            pt_pool.tile([128, 2, HBLK], BF16, tag="pt", name=f"pt{h}")
                        )
                    tunits = [(s, h2) for s in range(2) for h2 in range(0, H, 2)]

                gunits = [(s, g) for s in range(2) for g in range(4)]

                # q-units first (their outputs feed this hb's logits), then
                # softmax groups with the previous half-block's transposes
                # interleaved between them to keep PE fed during the softmax
                # chain latency
                for qu in qunits:
                    q_unit(*qu)
                ohatT = []
                ti = 0
                for s, g in gunits:
                    softmax_unit(hb, s, g)
                    for _ in range(2):
                        if ti < len(tunits):
                            t_unit(hbp, tunits[ti][0], tunits[ti][1], pts)
                            ti += 1
                while ti < len(tunits):
                    t_unit(hbp, tunits[ti][0], tunits[ti][1], pts)
                    ti += 1
                if hbp >= 0:
                    for j in range(8):
                        o_unit(j, pts, ohatT)
                    for s in range(2):
                        for half in range(2):
                            w_unit(hbp, s, half, ohatT)
                if hb >= 1 and hb % 2 == 0:
                    p_state.pop(((hb - 1) // 2, "qt"), None)
                if hb % 2 == 1 and hb > 0:
                    xt_state.pop(blk, None)

            def tail():
                hbp = NHB - 1
                pts = []
                for h in range(H):
                    pts.append(
                        pt_pool.tile([128, 2, HBLK], BF16, tag="pt", name=f"pt{h}")
                    )
                for s in range(2):
                    for h2 in range(0, H, 2):
                        t_unit(hbp, s, h2, pts)
                ohatT = []
                for j in range(8):
                    o_unit(j, pts, ohatT)
                for s in range(2):
                    for half in range(2):
                        w_unit(hbp, s, half, ohatT)

            for hb in range(NHB):
                emit_hb(hb)
            tail()

    nc.compile()
    return nc


def _prep_inputs(x, Wq, Wk, Wv, E, F, Wo, bo):
    x = np.asarray(x, dtype=np.float32)
    Wq = np.asarray(Wq, dtype=np.float32)
    Wk = np.asarray(Wk, dtype=np.float32)
    Wv = np.asarray(Wv, dtype=np.float32)
    E = np.asarray(E, dtype=np.float32)
    F_ = np.asarray(F, dtype=np.float32)
    Wo = np.asarray(Wo, dtype=np.float32)

    xsum = x.sum(axis=1)                     # (B, D)
    S_k = xsum @ Wk.T                        # (B, D)
    S_v = xsum @ Wv.T                        # (B, D)

    wqT = _round_fp32r(np.ascontiguousarray(Wq.T))
    woT = np.ascontiguousarray(Wo.T).astype(ml_dtypes.bfloat16)
    ident = np.eye(128, dtype=ml_dtypes.bfloat16)

    in_maps = []
    for core in range(NCORES):
        b, half = core // 2, core % 2
        xs = x[b, half * NH:(half + 1) * NH, :]          # (NH, D)
        xT = _round_fp32r(np.ascontiguousarray(xs.T))    # (D, NH)

        # E-hat: block-diagonal per head pair -> one (128,512) rhs per pair
        ehat = np.zeros((128, 8, 2 * KP), dtype=np.float32)
        for h in range(H):
            sk = S_k[b, h * HD:(h + 1) * HD]             # (64,)
            j, hh = h // 2, h % 2
            ehat[hh * 64:hh * 64 + 64, j, hh * KP:(hh + 1) * KP] = (E.T * sk[:, None]) / 8.0
        ehat = _round_fp32r(ehat)

        # F-hat: block-diagonal pair packing, (128, pair, head-in-pair, chunk, 64*2)
        fhat = np.zeros((128, 8, 2, 2, 128), dtype=np.float32)
        for h in range(H):
            sv = S_v[b, h * HD:(h + 1) * HD]             # (64,)
            fh = F_ * sv[None, :]                        # (KP, 64)
            j, hh = h // 2, h % 2
            for c in range(2):
                fhat[:, j, hh, c, hh * 64:(hh + 1) * 64] = fh[c * 128:(c + 1) * 128, :]
        fhat = fhat.astype(ml_dtypes.bfloat16)

        in_maps.append({
            "xT": xT, "wqT": wqT, "woT": woT, "ehat": ehat,
            "fhat": fhat, "ident": ident,
        })
    return in_maps


def _run(inputs: dict, trace: bool = False, tmpdir: str | None = None):
    if "nc" not in _CACHE:
        _CACHE["nc"] = _build()
    nc = _CACHE["nc"]
    bo = np.asarray(inputs["bo"], dtype=np.float32)
    in_maps = _prep_inputs(**inputs)
    res = bass_utils.run_bass_kernel_spmd(
        nc, in_maps, core_ids=list(range(NCORES)), trace=trace, tmpdir=tmpdir
    )
    out = np.empty((B, N, D), dtype=np.float32)
    for core in range(NCORES):
        b, half = core // 2, core % 2
        out[b, half * NH:(half + 1) * NH, :] = (
            res.results[core]["out"].astype(np.float32) + bo[None, :]
        )
    return out, res


def kernel(**inputs) -> np.ndarray:
    out, _ = _run(inputs)
    return out
